# revision 4
# baseline (speedup 1.0000x reference)
"""HardL1ACELoss (20-bin calibration histogram) on 8 TRN2 NeuronCores.

Strategy: data-parallel shard of the flat 32M stream. Per core, encode each
element's (bin, target) as a small integer y = bin + 120*target via one fp32
multiply-subtract (RNE fp32->int16 on DVE, verified on HW) and one
scalar-tensor-tensor op. All per-bin statistics are then cumulative-threshold
reductions:
  C_j  = #{bin >= j}               (int16 is_ge at DVE 4x rate, accum_out)
  T_j  = #{target=1 and bin >= j}  (same, thresholds 120+j; some on ACT Sign)
  M_j  = sum max(p_bf16, t_j)      (bf16 max at DVE 4x rate, accum_out)
giving S_j = sum p[p>=t_j] = M_j - t_j*(N - C_j). Per-bin sums/counts are
adjacent differences; the tiny final algebra runs on host in fp64.
"""
import numpy as np

N_BINS = 20
NCORES = 8
N = 33554432
PER = N // NCORES          # 4194304 per core
P = 128
FREE = PER // P            # 32768 fp32 per partition
F = 2048                   # tile free-dim
NTILES = FREE // F         # 16
C_NUDGE = 0.4999999701976776  # RNE(20p - C_NUDGE) == floor(20p) with safe boundaries
BINS = np.linspace(0.0, 1.0, N_BINS + 1).astype(np.float32)

N_C = 19                   # C_j, j=1..19
N_M = 20                   # M_j, j=0..19
T_DVE = list(range(15, 20))   # T_j on DVE
T_ACT = list(range(1, 15))    # T_j on ACT (Sign)
NQ_DVE = N_C + 1 + N_M + len(T_DVE)   # C's, N1, M's, T's = 45
NQ_ACT = len(T_ACT)                   # 14

_CACHE = {}


def _build():
    import concourse.bass as bass
    import concourse.tile as tile
    from concourse import bacc, mybir

    nc = bacc.Bacc("TRN2", target_bir_lowering=False, debug=False,
                   enable_asserts=False, num_devices=NCORES)
    p_d = nc.dram_tensor("p", [P, FREE], mybir.dt.float32, kind="ExternalInput")
    t_d = nc.dram_tensor("t", [P, FREE], mybir.dt.int32, kind="ExternalInput")
    accd_d = nc.dram_tensor("acc_dve", [P, NQ_DVE * NTILES], mybir.dt.float32,
                            kind="ExternalOutput")
    acca_d = nc.dram_tensor("acc_act", [P, NQ_ACT * NTILES], mybir.dt.float32,
                            kind="ExternalOutput")
    A = mybir.AluOpType

    with tile.TileContext(nc) as tc:
        with tc.tile_pool(name="io", bufs=3) as io_pool, \
             tc.tile_pool(name="enc", bufs=2) as enc_pool, \
             tc.tile_pool(name="scr", bufs=2) as scr_pool, \
             tc.tile_pool(name="acc", bufs=1) as acc_pool:
            acc_dve = acc_pool.tile([P, NQ_DVE * NTILES], mybir.dt.float32)
            acc_act = acc_pool.tile([P, NQ_ACT * NTILES], mybir.dt.float32)
            bias_act = acc_pool.tile([P, NQ_ACT], mybir.dt.float32)
            for k, j in enumerate(T_ACT):
                nc.vector.memset(bias_act[:, k:k + 1], -(119.5 + j))

            for i in range(NTILES):
                pt = io_pool.tile([P, F], mybir.dt.float32, tag="p")
                nc.sync.dma_start(pt[:], p_d.ap()[:, bass.ts(i, F)])
                tt = io_pool.tile([P, F], mybir.dt.int32, tag="t")
                nc.sync.dma_start(tt[:], t_d.ap()[:, bass.ts(i, F)])

                # y0 = RNE(20p - c) = bin index, int16 (HW-verified RNE)
                y0 = enc_pool.tile([P, F], mybir.dt.int16, tag="y0")
                nc.vector.tensor_scalar(out=y0[:], in0=pt[:], scalar1=20.0,
                                        scalar2=C_NUDGE, op0=A.mult,
                                        op1=A.subtract)
                # y = y0 + 120*t
                y = enc_pool.tile([P, F], mybir.dt.int16, tag="y")
                nc.vector.scalar_tensor_tensor(out=y[:], in0=tt[:], scalar=120.0,
                                               in1=y0[:], op0=A.mult, op1=A.add)
                # pb = bf16(p)
                pb = enc_pool.tile([P, F], mybir.dt.bfloat16, tag="pb")
                nc.vector.tensor_copy(pb[:], pt[:])

                sc16 = scr_pool.tile([P, F], mybir.dt.int16, tag="sc16")
                scbf = scr_pool.tile([P, F], mybir.dt.bfloat16, tag="scbf")
                scact = scr_pool.tile([P, F], mybir.dt.bfloat16, tag="scact")

                q = 0
                # C_j = count(y0 >= j), j=1..19  (DVE int16 4x)
                for j in range(1, 20):
                    nc.vector.tensor_scalar(
                        out=sc16[:], in0=y0[:], scalar1=float(j) - 0.5,
                        scalar2=0.0, op0=A.is_ge, op1=A.add,
                        accum_out=acc_dve[:, q * NTILES + i: q * NTILES + i + 1])
                    q += 1
                # N1 = count(y >= 60)
                nc.vector.tensor_scalar(
                    out=sc16[:], in0=y[:], scalar1=60.0, scalar2=0.0,
                    op0=A.is_ge, op1=A.add,
                    accum_out=acc_dve[:, q * NTILES + i: q * NTILES + i + 1])
                q += 1
                # M_j = sum max(pb, BINS[j]), j=0..19  (DVE bf16 4x)
                for j in range(20):
                    nc.vector.tensor_scalar(
                        out=scbf[:], in0=pb[:], scalar1=float(BINS[j]),
                        scalar2=0.0, op0=A.max, op1=A.add,
                        accum_out=acc_dve[:, q * NTILES + i: q * NTILES + i + 1])
                    q += 1
                # T_j = count(y >= 120+j) for j in T_DVE
                for j in T_DVE:
                    nc.vector.tensor_scalar(
                        out=sc16[:], in0=y[:], scalar1=119.5 + j, scalar2=0.0,
                        op0=A.is_ge, op1=A.add,
                        accum_out=acc_dve[:, q * NTILES + i: q * NTILES + i + 1])
                    q += 1
                assert q == NQ_DVE

                # ACT: sign sums -> T_j for j in T_ACT
                for k, j in enumerate(T_ACT):
                    nc.scalar.activation(
                        out=scact[:], in_=y[:],
                        func=mybir.ActivationFunctionType.Sign,
                        bias=bias_act[:, k:k + 1], scale=1.0,
                        accum_out=acc_act[:, k * NTILES + i: k * NTILES + i + 1])

            nc.sync.dma_start(accd_d.ap()[:], acc_dve[:])
            nc.sync.dma_start(acca_d.ap()[:], acc_act[:])
    nc.compile()
    return nc


def _get_nc():
    if "nc" not in _CACHE:
        _CACHE["nc"] = _build()
    return _CACHE["nc"]


def kernel(preds, targets):
    from concourse.bass_utils import run_bass_kernel_spmd

    preds = np.ascontiguousarray(np.asarray(preds, dtype=np.float32).reshape(-1))
    targets = np.ascontiguousarray(np.asarray(targets, dtype=np.int32).reshape(-1))
    assert preds.shape == (N,) and targets.shape == (N,)

    nc = _get_nc()
    in_maps = [
        {"p": preds[c * PER:(c + 1) * PER].reshape(P, FREE),
         "t": targets[c * PER:(c + 1) * PER].reshape(P, FREE)}
        for c in range(NCORES)
    ]
    import time as _time
    _t0 = _time.time()
    res = run_bass_kernel_spmd(nc, in_maps, core_ids=list(range(NCORES)))
    _CACHE["spmd_wall_s"] = _time.time() - _t0
    _CACHE["last_results"] = res

    qd = np.zeros(NQ_DVE, dtype=np.float64)
    qa = np.zeros(NQ_ACT, dtype=np.float64)
    for c in range(NCORES):
        r = res.results[c]
        qd += r["acc_dve"].astype(np.float64).sum(axis=0).reshape(NQ_DVE, NTILES).sum(axis=1)
        qa += r["acc_act"].astype(np.float64).sum(axis=0).reshape(NQ_ACT, NTILES).sum(axis=1)

    C = np.zeros(21); C[0] = float(N); C[1:20] = qd[0:19]
    N1 = qd[19]
    M = qd[20:40]
    T = np.zeros(21); T[0] = N1
    for k, j in enumerate(T_DVE):
        T[j] = qd[40 + k]
    for k, j in enumerate(T_ACT):
        T[j] = (qa[k] + float(N)) / 2.0   # sign-sum -> count
    S = np.zeros(21)
    tj = BINS.astype(np.float64)
    S[0:20] = M - tj[0:20] * (float(N) - C[0:20])
    cnt = C[0:20] - C[1:21]
    st = T[0:20] - T[1:21]
    sp = S[0:20] - S[1:21]
    nonempty = cnt > 0
    safe = np.where(nonempty, cnt, 1.0)
    ace = np.sum(np.where(nonempty, np.abs(sp / safe - st / safe), 0.0)) / N_BINS
    return np.float32(ace)
